# revision 5
# baseline (speedup 1.0000x reference)
"""MoE layer (8 experts, top-2) on 8 TRN2 NeuronCores.

Sharding: data-parallel over the token axis. Each core gets T/8 = 512
tokens (pre-transposed to [D, 512] on host) plus a replicated copy of
gate_w / w1 / w2 (w1/w2 pre-tiled on host so every weight DMA is a
fully-contiguous block). Each core computes the gate, top-2 routing and
softmax on device, folds the per-token combine weight into the moving
operand of the first expert matmul (relu commutes with positive row
scaling), runs the dense per-expert FFN in float32r, and accumulates
all experts into out^T [D, 512]. Host unshard = transpose+concat.
"""

import os

import numpy as np

NUM_EXPERTS = 8
TOP_K = 2
D = 1024
F = 4096
B, S = 2, 2048
T = B * S
N_CORES = 8
TPC = T // N_CORES  # tokens per core = 512

LAST_RESULT = None  # BassKernelResults of the most recent run (for test harness)


def _build_nc():
    import concourse.mybir as mybir
    import concourse.tile as tile
    from concourse import bacc
    from concourse.masks import make_identity

    dt = mybir.dt
    nc = bacc.Bacc("TRN2", target_bir_lowering=False, debug=False, num_devices=N_CORES)

    xT_d = nc.dram_tensor("xT", [D, TPC], dt.float32, kind="ExternalInput").ap()
    gw_d = nc.dram_tensor("gate_w", [D, NUM_EXPERTS], dt.float32, kind="ExternalInput").ap()
    # w1p[e, fc, p, ko, f]: w1[e, ko*128+p, fc*512+f]
    w1_d = nc.dram_tensor("w1p", [8, 16, 128, 8, 256], dt.float32r, kind="ExternalInput").ap()
    # w2p[e, dm, p, ko, m]: w2[e, ko*128+p, dm*128+m]
    w2_d = nc.dram_tensor("w2p", [8, 8, 128, 32, 128], dt.float32r, kind="ExternalInput").ap()
    outT_d = nc.dram_tensor("outT", [D, TPC], dt.float32, kind="ExternalOutput").ap()

    with tile.TileContext(nc) as tc:
        with (
            tc.tile_pool(name="resident", bufs=1) as res,
            tc.tile_pool(name="w1pool", bufs=2) as w1pool,
            tc.tile_pool(name="w2pool", bufs=2) as w2pool,
            tc.tile_pool(name="route", bufs=1) as route,
            tc.tile_pool(name="cbpool", bufs=2) as cbpool,
            tc.tile_pool(name="psum_g", bufs=2, space="PSUM") as psum_g,
            tc.tile_pool(name="psum_t", bufs=2, space="PSUM") as psum_t,
            tc.tile_pool(name="psum_h", bufs=2, space="PSUM") as psum_h,
            tc.tile_pool(name="psum_o", bufs=2, space="PSUM") as psum_o,
        ):
            # ---- resident loads -------------------------------------------------
            XT = res.tile([128, 8, TPC], dt.float32)  # x^T, K-major tiles
            nc.sync.dma_start(XT[:], xT_d.rearrange("(o p) t -> p o t", p=128))
            GW = res.tile([128, 8, NUM_EXPERTS], dt.float32)
            nc.sync.dma_start(GW[:], gw_d.rearrange("(o p) e -> p o e", p=128))

            ident = res.tile([128, 128], dt.float32)
            make_identity(nc, ident)

            # ---- gate logits [512, 8] ------------------------------------------
            LG = route.tile([128, 4, NUM_EXPERTS], dt.float32)  # token = mt*128+p
            for mt in range(4):
                pg = psum_g.tile([128, NUM_EXPERTS], dt.float32)
                for ko in range(8):
                    nc.tensor.matmul(
                        pg[:],
                        XT[:, ko, mt * 128 : (mt + 1) * 128],
                        GW[:, ko, :],
                        start=(ko == 0),
                        stop=(ko == 7),
                    )
                nc.vector.tensor_copy(LG[:, mt, :], pg[:])

            # ---- top-2 + softmax -> combine [512, 8] ---------------------------
            au = mybir.AluOpType
            M1 = route.tile([128, 4], dt.float32)
            M2 = route.tile([128, 4], dt.float32)
            MK1 = route.tile([128, 4, NUM_EXPERTS], dt.float32)
            MK2 = route.tile([128, 4, NUM_EXPERTS], dt.float32)
            LG2 = route.tile([128, 4, NUM_EXPERTS], dt.float32)
            DD = route.tile([128, 4], dt.float32)
            P1 = route.tile([128, 4], dt.float32)
            P2 = route.tile([128, 4], dt.float32)
            TM1 = route.tile([128, 4, NUM_EXPERTS], dt.float32)
            COMB = route.tile([128, 4, NUM_EXPERTS], dt.float32)

            sh = [128, 4, NUM_EXPERTS]
            nc.vector.tensor_reduce(M1[:], LG[:], mybir.AxisListType.X, au.max)
            nc.vector.tensor_tensor(
                MK1[:], LG[:], M1[:, :, None].to_broadcast(sh), au.is_equal
            )
            nc.vector.scalar_tensor_tensor(
                LG2[:], MK1[:], -1e30, LG[:], au.mult, au.add
            )
            nc.vector.tensor_reduce(M2[:], LG2[:], mybir.AxisListType.X, au.max)
            nc.vector.tensor_tensor(
                MK2[:], LG2[:], M2[:, :, None].to_broadcast(sh), au.is_equal
            )
            nc.vector.tensor_tensor(DD[:], M1[:], M2[:], au.subtract)
            nc.scalar.activation(P1[:], DD[:], mybir.ActivationFunctionType.Sigmoid)
            nc.vector.tensor_scalar(P2[:], P1[:], -1.0, 1.0, au.mult, au.add)
            nc.vector.tensor_tensor(
                TM1[:], MK1[:], P1[:, :, None].to_broadcast(sh), au.mult
            )
            nc.vector.tensor_tensor(
                COMB[:], MK2[:], P2[:, :, None].to_broadcast(sh), au.mult
            )
            nc.vector.tensor_tensor(COMB[:], TM1[:], COMB[:], au.add)

            # ---- per-expert combine rows on partition 0, then broadcast --------
            CTE = route.tile([1, NUM_EXPERTS, TPC], dt.float32)
            for e in range(NUM_EXPERTS):
                pt = psum_t.tile([1, TPC], dt.float32)
                for mt in range(4):
                    nc.tensor.transpose(
                        pt[:, mt * 128 : (mt + 1) * 128],
                        COMB[:, mt, e : e + 1],
                        ident[:],
                    )
                nc.vector.tensor_copy(CTE[:, e, :], pt[:])

            # ---- expert loop ----------------------------------------------------
            ACC = res.tile([128, 8, TPC], dt.float32)  # out^T accumulator
            H = res.tile([128, 32, TPC], dt.float32r)  # h^T for one expert

            for e in range(NUM_EXPERTS):
                CB = cbpool.tile([128, TPC], dt.float32, tag="CB")
                nc.gpsimd.partition_broadcast(CB[:], CTE[:, e, :])
                RH = res.tile([128, 8, TPC], dt.float32r, tag="RH")
                nc.vector.tensor_tensor(
                    RH[:],
                    XT[:],
                    CB[:, None, :].to_broadcast([128, 8, TPC]),
                    au.mult,
                )
                # mm1: h^T[F, T] = relu(w1^T @ (x^T * c_e))
                for fc in range(16):
                    W1C = w1pool.tile([128, 8, 256], dt.float32r, tag="w1c")
                    nc.sync.dma_start(W1C[:], w1_d[e, fc])
                    for fs in range(2):
                        ph = psum_h.tile([128, TPC], dt.float32)
                        for ko in range(8):
                            nc.tensor.matmul(
                                ph[:],
                                W1C[:, ko, fs * 128 : (fs + 1) * 128],
                                RH[:, ko, :],
                                start=(ko == 0),
                                stop=(ko == 7),
                            )
                        nc.scalar.activation(
                            H[:, fc * 2 + fs, :],
                            ph[:],
                            mybir.ActivationFunctionType.Relu,
                        )
                # mm2: out^T[D, T] += w2^T @ h^T
                for dm in range(8):
                    W2C = w2pool.tile([128, 32, 128], dt.float32r, tag="w2c")
                    nc.sync.dma_start(W2C[:], w2_d[e, dm])
                    po = psum_o.tile([128, TPC], dt.float32)
                    for ko in range(32):
                        nc.tensor.matmul(
                            po[:],
                            W2C[:, ko, :],
                            H[:, ko, :],
                            start=(ko == 0),
                            stop=(ko == 31),
                        )
                    if e == 0:
                        nc.vector.tensor_copy(ACC[:, dm, :], po[:])
                    else:
                        nc.vector.tensor_tensor(ACC[:, dm, :], po[:], ACC[:, dm, :], au.add)

            # ---- store out^T ----------------------------------------------------
            nc.sync.dma_start(outT_d.rearrange("(o p) t -> p o t", p=128), ACC[:])

    nc.compile()
    return nc


def kernel(hidden_states, gate_w, w1, w2):
    global LAST_RESULT
    from concourse.bass_utils import run_bass_kernel_spmd

    x = np.ascontiguousarray(np.asarray(hidden_states, dtype=np.float32)).reshape(T, D)
    gw = np.ascontiguousarray(np.asarray(gate_w, dtype=np.float32))
    w1n = np.asarray(w1, dtype=np.float32)
    w2n = np.asarray(w2, dtype=np.float32)

    # pre-tiled weight layouts (every DMA chunk contiguous)
    w1p = np.ascontiguousarray(
        w1n.reshape(8, 8, 128, 16, 256).transpose(0, 3, 2, 1, 4)
    )
    w2p = np.ascontiguousarray(
        w2n.reshape(8, 32, 128, 8, 128).transpose(0, 3, 2, 1, 4)
    )

    nc = _build_nc()

    in_maps = []
    for c in range(N_CORES):
        xTc = np.ascontiguousarray(x[c * TPC : (c + 1) * TPC].T)
        in_maps.append({"xT": xTc, "gate_w": gw, "w1p": w1p, "w2p": w2p})

    trace = bool(os.environ.get("MOE_TRACE"))
    LAST_RESULT = run_bass_kernel_spmd(
        nc, in_maps, core_ids=list(range(N_CORES)), trace=trace
    )

    out = np.empty((T, D), dtype=np.float32)
    for c in range(N_CORES):
        out[c * TPC : (c + 1) * TPC] = LAST_RESULT.results[c]["outT"].T
    return out.reshape(B, S, D)
